# revision 3
# baseline (speedup 1.0000x reference)
"""DCN (Deep & Cross Network) forward pass on 8 Trainium2 NeuronCores.

Strategy: data-parallel over the batch (16384 / 8 = 2048 rows per core);
embedding tables replicated per core (HBM-resident; the kernel only reads
the gathered rows, ~4 MB/core, via indirect DMA).

Math: with cross layers of the form x_{i+1} = x0 * (x_i @ w) + x_i the
cross stack collapses per row to x2 = x0 * (1 + x0@cw1) * (1 + x0@cw2),
so  out = (1+s)(1+t) * (x0 @ Wo[:512]) + relu_mlp(x0) @ Wo[512:] + bo
with s = x0@cw1, t = x0@cw2.  The kernel computes s, t, u = x0@Wo[:512]
as one 3-column matmul and never materializes x1/x2.

Layout per 128-row tile:
  gather user+item rows -> X0 [128, 512] (batch on partitions)
  PE-transpose -> X0T [128, 512] = 4 chunks [feat128, batch128]
  MLP runs feature-major (weights stationary, batch on the free dim) so
  no further transposes are needed; biases become per-partition scalars.
"""

import numpy as np

B = 16384
NCORES = 8
BC = B // NCORES        # 2048 rows per core
P = 128
NT = BC // P            # 16 tiles per core
D = 512
H = D // 2              # 256, per-table embedding dim
USER_NUM = 500000
ITEM_NUM = 100000

_CACHE = {}


def _build_nc():
    import sys
    if "/opt/trn_rl_repo" not in sys.path:
        sys.path.insert(0, "/opt/trn_rl_repo")
    import concourse.bass as bass
    import concourse.tile as tile
    from concourse import bacc, mybir
    from concourse.masks import make_identity
    from contextlib import ExitStack

    f32 = mybir.dt.float32
    i32 = mybir.dt.int32

    nc = bacc.Bacc("TRN2", target_bir_lowering=False, debug=False)

    uemb = nc.dram_tensor("uemb", [USER_NUM, H], f32, kind="ExternalInput")
    iemb = nc.dram_tensor("iemb", [ITEM_NUM, H], f32, kind="ExternalInput")
    uidx = nc.dram_tensor("uidx", [P, NT], i32, kind="ExternalInput")
    iidx = nc.dram_tensor("iidx", [P, NT], i32, kind="ExternalInput")
    # dw1 packed [128, 4*2*128]: col (c*256 + m*128 + j) = dW1[c*128+p, m*128+j]
    dw1 = nc.dram_tensor("dw1", [P, 1024], f32, kind="ExternalInput")
    dw2 = nc.dram_tensor("dw2", [P, 256], f32, kind="ExternalInput")
    dw3 = nc.dram_tensor("dw3", [P, 64], f32, kind="ExternalInput")
    # cwm packed [128, 4*3]: col (c*3 + j) = [cw1, cw2, Wo[:512]][c*128+p, j]
    cwm = nc.dram_tensor("cwm", [P, 12], f32, kind="ExternalInput")
    db1 = nc.dram_tensor("db1", [P, 2], f32, kind="ExternalInput")
    db2 = nc.dram_tensor("db2", [P, 1], f32, kind="ExternalInput")
    db3 = nc.dram_tensor("db3", [64, 1], f32, kind="ExternalInput")
    woh = nc.dram_tensor("woh", [64, 1], f32, kind="ExternalInput")
    bo = nc.dram_tensor("bo", [P, 1], f32, kind="ExternalInput")
    out = nc.dram_tensor("out", [NT, P], f32, kind="ExternalOutput")

    with tile.TileContext(nc) as tc, ExitStack() as ctx:
        cb = ctx.enter_context(tc.tile_pool(name="consts", bufs=1))
        gp = ctx.enter_context(tc.tile_pool(name="gather", bufs=3))
        xp = ctx.enter_context(tc.tile_pool(name="x0t", bufs=3))
        h1p = ctx.enter_context(tc.tile_pool(name="h1", bufs=2))
        h2p = ctx.enter_context(tc.tile_pool(name="h2", bufs=2))
        h3p = ctx.enter_context(tc.tile_pool(name="h3", bufs=2))
        ps_t = ctx.enter_context(tc.tile_pool(name="ps_t", bufs=2, space="PSUM"))
        ps_h1 = ctx.enter_context(tc.tile_pool(name="ps_h1", bufs=2, space="PSUM"))
        ps_h2 = ctx.enter_context(tc.tile_pool(name="ps_h2", bufs=1, space="PSUM"))
        ps_h3 = ctx.enter_context(tc.tile_pool(name="ps_h3", bufs=1, space="PSUM"))
        ps_sw = ctx.enter_context(tc.tile_pool(name="ps_sw", bufs=1, space="PSUM"))

        ident = cb.tile([P, P], f32)
        make_identity(nc, ident[:])

        uidx_sb = cb.tile([P, NT], i32)
        nc.sync.dma_start(out=uidx_sb[:], in_=uidx[:, :])
        iidx_sb = cb.tile([P, NT], i32)
        nc.sync.dma_start(out=iidx_sb[:], in_=iidx[:, :])
        dw1_sb = cb.tile([P, 1024], f32)
        nc.sync.dma_start(out=dw1_sb[:], in_=dw1[:, :])
        dw2_sb = cb.tile([P, 256], f32)
        nc.sync.dma_start(out=dw2_sb[:], in_=dw2[:, :])
        dw3_sb = cb.tile([P, 64], f32)
        nc.sync.dma_start(out=dw3_sb[:], in_=dw3[:, :])
        cwm_sb = cb.tile([P, 12], f32)
        nc.sync.dma_start(out=cwm_sb[:], in_=cwm[:, :])
        db1_sb = cb.tile([P, 2], f32)
        nc.sync.dma_start(out=db1_sb[:], in_=db1[:, :])
        db2_sb = cb.tile([P, 1], f32)
        nc.sync.dma_start(out=db2_sb[:], in_=db2[:, :])
        db3_sb = cb.tile([64, 1], f32)
        nc.sync.dma_start(out=db3_sb[:], in_=db3[:, :])
        woh_sb = cb.tile([64, 1], f32)
        nc.sync.dma_start(out=woh_sb[:], in_=woh[:, :])
        bo_sb = cb.tile([P, 1], f32)
        nc.sync.dma_start(out=bo_sb[:], in_=bo[:, :])

        # per-tile s,t,u,w accumulated here; columns [4t : 4t+4]
        sw_all = cb.tile([P, 4 * NT], f32)

        Relu = mybir.ActivationFunctionType.Relu

        for t in range(NT):
            g = gp.tile([P, D], f32)
            nc.gpsimd.indirect_dma_start(
                out=g[:, 0:H], out_offset=None, in_=uemb[:, :],
                in_offset=bass.IndirectOffsetOnAxis(ap=uidx_sb[:, t : t + 1], axis=0),
            )
            nc.gpsimd.indirect_dma_start(
                out=g[:, H:D], out_offset=None, in_=iemb[:, :],
                in_offset=bass.IndirectOffsetOnAxis(ap=iidx_sb[:, t : t + 1], axis=0),
            )

            # transpose all 4 chunks into one PSUM bank, one copy out
            pt = ps_t.tile([P, D], f32, space="PSUM", tag="pt")
            for c in range(4):
                nc.tensor.transpose(
                    out=pt[:, c * P : (c + 1) * P],
                    in_=g[:, c * P : (c + 1) * P],
                    identity=ident[:],
                )
            x0t = xp.tile([P, D], f32)
            nc.vector.tensor_copy(out=x0t[:], in_=pt[:])

            # s,t,u = x0 @ [cw1, cw2, Wo_x]  -> psum [P(batch), 3]
            psw = ps_sw.tile([P, 4], f32, space="PSUM")
            for c in range(4):
                nc.tensor.matmul(
                    out=psw[:, 0:3],
                    lhsT=x0t[:, c * P : (c + 1) * P],
                    rhs=cwm_sb[:, 3 * c : 3 * c + 3],
                    start=(c == 0), stop=(c == 3),
                )

            # h1T = relu(dW1^T @ x0^T + db1)  [256 feat -> 2 chunks, 128 batch]
            h1 = h1p.tile([P, 2 * P], f32)
            for m in range(2):
                ph1 = ps_h1.tile([P, P], f32, space="PSUM")
                for c in range(4):
                    nc.tensor.matmul(
                        out=ph1[:],
                        lhsT=dw1_sb[:, c * 256 + m * P : c * 256 + (m + 1) * P],
                        rhs=x0t[:, c * P : (c + 1) * P],
                        start=(c == 0), stop=(c == 3),
                    )
                nc.scalar.activation(
                    out=h1[:, m * P : (m + 1) * P], in_=ph1[:], func=Relu,
                    bias=db1_sb[:, m : m + 1],
                )

            # h2T = relu(dW2^T @ h1T + db2)  [128 feat, 128 batch]
            ph2 = ps_h2.tile([P, P], f32, space="PSUM")
            for m in range(2):
                nc.tensor.matmul(
                    out=ph2[:],
                    lhsT=dw2_sb[:, m * P : (m + 1) * P],
                    rhs=h1[:, m * P : (m + 1) * P],
                    start=(m == 0), stop=(m == 1),
                )
            h2 = h2p.tile([P, P], f32)
            nc.scalar.activation(out=h2[:], in_=ph2[:], func=Relu, bias=db2_sb[:])

            # h3T = relu(dW3^T @ h2T + db3)  [64 feat, 128 batch]
            ph3 = ps_h3.tile([64, P], f32, space="PSUM")
            nc.tensor.matmul(out=ph3[:], lhsT=dw3_sb[:], rhs=h2[:], start=True, stop=True)
            h3 = h3p.tile([64, P], f32)
            nc.scalar.activation(out=h3[:], in_=ph3[:], func=Relu, bias=db3_sb[:])

            # w = h3 . Wo[512:]  -> psum [P(batch), 1] in column 3
            nc.tensor.matmul(out=psw[:, 3:4], lhsT=h3[:], rhs=woh_sb[:], start=True, stop=True)

            nc.vector.tensor_copy(out=sw_all[:, 4 * t : 4 * t + 4], in_=psw[:])

        # final combine: r = (1+s)(1+t)*u + w + bo   on [128, 16]
        s_v = sw_all[:, 0 :: 4]
        t_v = sw_all[:, 1 :: 4]
        u_v = sw_all[:, 2 :: 4]
        w_v = sw_all[:, 3 :: 4]
        f1 = cb.tile([P, NT], f32)
        f2 = cb.tile([P, NT], f32)
        nc.vector.tensor_scalar_add(out=f1[:], in0=s_v, scalar1=1.0)
        nc.vector.tensor_scalar_add(out=f2[:], in0=t_v, scalar1=1.0)
        nc.vector.tensor_mul(out=f1[:], in0=f1[:], in1=f2[:])
        nc.vector.tensor_mul(out=f1[:], in0=f1[:], in1=u_v)
        nc.vector.tensor_add(out=f1[:], in0=f1[:], in1=w_v)
        nc.vector.tensor_scalar_add(out=f1[:], in0=f1[:], scalar1=bo_sb[:, 0:1])

        # transpose [128, 16] -> [16, 128] so the output DMA is contiguous
        pr = ps_t.tile([P, D], f32, space="PSUM", tag="pt")
        nc.tensor.transpose(out=pr[:NT, :P], in_=f1[:, :], identity=ident[:])
        res = cb.tile([NT, P], f32)
        nc.vector.tensor_copy(out=res[:], in_=pr[:NT, :P])
        nc.sync.dma_start(out=out[:, :], in_=res[:])

    nc.compile()
    return nc


def _get_nc():
    if "nc" not in _CACHE:
        _CACHE["nc"] = _build_nc()
    return _CACHE["nc"]


def kernel(user, item, user_emb, item_emb, cw1, cw2, dW1, db1, dW2, db2, dW3, db3, Wo, bo):
    import sys
    if "/opt/trn_rl_repo" not in sys.path:
        sys.path.insert(0, "/opt/trn_rl_repo")
    from concourse.bass_utils import run_bass_kernel_spmd

    user = np.asarray(user)
    item = np.asarray(item)
    f = lambda a: np.ascontiguousarray(np.asarray(a), dtype=np.float32)
    user_emb = f(user_emb)
    item_emb = f(item_emb)
    cw1, cw2 = f(cw1), f(cw2)
    dW1, db1 = f(dW1), f(db1)
    dW2, db2 = f(dW2), f(db2)
    dW3, db3 = f(dW3), f(db3)
    Wo, bo = f(Wo), f(bo)

    shared = {
        "uemb": user_emb,
        "iemb": item_emb,
        "dw1": np.ascontiguousarray(
            dW1.reshape(4, P, 2, P).transpose(1, 0, 2, 3).reshape(P, 1024)
        ),
        "dw2": np.ascontiguousarray(
            dW2.reshape(2, P, P).transpose(1, 0, 2).reshape(P, 256)
        ),
        "dw3": np.ascontiguousarray(dW3),
        "cwm": np.ascontiguousarray(
            np.stack([cw1, cw2, Wo[:D, 0]], axis=1).reshape(4, P, 3)
            .transpose(1, 0, 2).reshape(P, 12)
        ),
        "db1": np.ascontiguousarray(db1.reshape(2, P).T),
        "db2": np.ascontiguousarray(db2.reshape(P, 1)),
        "db3": np.ascontiguousarray(db3.reshape(64, 1)),
        "woh": np.ascontiguousarray(Wo[D:].reshape(64, 1)),
        "bo": np.full((P, 1), float(bo.reshape(-1)[0]), np.float32),
    }
    uidx = user.astype(np.int32).reshape(NCORES, NT, P)
    iidx = item.astype(np.int32).reshape(NCORES, NT, P)
    in_maps = []
    for c in range(NCORES):
        m = dict(shared)
        m["uidx"] = np.ascontiguousarray(uidx[c].T)
        m["iidx"] = np.ascontiguousarray(iidx[c].T)
        in_maps.append(m)

    nc = _get_nc()
    res = run_bass_kernel_spmd(nc, in_maps, list(range(NCORES)))
    outs = [res.results[c]["out"].reshape(-1) for c in range(NCORES)]
    return np.concatenate(outs)
